# revision 1
# baseline (speedup 1.0000x reference)
"""Trainium2 Bass kernel for nn_ButterflyFFT (Monarch butterfly, N=4096, B=8192).

Math (per batch row b, viewing x[b] as a 64x64 matrix X with X[p,k]=x[b,p*64+k]):
  stage 1: for each column k: Y[:,k] = w1c[k] @ X[:,k]       (64x64 complex, X real)
  stage 2: for each row    l: Z[l,:] = w2c[l] @ Y[l,:]       (64x64 complex)
  output:  out[b, s*64+l] = Z[l,s]                            (complex64)

Device pipeline per core (B_core=1024, supertiles of BT=256):
  1. gather-DMA x -> T1[(h,p), (b0,k)] fp16 (cast in SWDGE DMA)
  2. stage 1, data-stationary fp16 matmuls: out (b, q2) -> G[b, ch, q*128+c*64+r]
  3. PE transpose per (l, ch): G-slice (b, (c r)) -> T2 (rc, b)
  4. stage 2, weights-stationary fp16 matmuls: O2 (c's*64+s, b)
  5. DMA out fp16 (l, cs, b); host reassembles complex64.
"""

import numpy as np

N = 4096
B = 8192
NCORES = 8
B_CORE = B // NCORES  # 1024
BT = 256              # supertile batch
NT = B_CORE // BT     # 4 supertiles
F16 = np.float16


def _build_host_weights(w1_bfly: np.ndarray, w2_bfly: np.ndarray):
    """W1all[64h+p, k*128 + c*64 + q] = w1_bfly[k,q,p,c]  (dup across h)
       W2all[c*64+r, l*128 + c'*64 + s] = stage-2 complex-matmul real form."""
    w1 = w1_bfly.astype(np.float32)              # (k, q, p, c)
    W1 = np.transpose(w1, (2, 0, 3, 1))          # (p, k, c, q)
    W1 = W1.reshape(64, 64 * 128).astype(F16)    # [p, k*128 + c*64 + q]
    W1all = np.concatenate([W1, W1], axis=0)     # dup rows for h=0/1

    w2r = w2_bfly[..., 0].astype(np.float32)     # (l, s, r)
    w2i = w2_bfly[..., 1].astype(np.float32)
    W2 = np.empty((2, 64, 64, 2, 64), dtype=np.float32)  # [c, r, l, c', s]
    W2[0, :, :, 0, :] = np.transpose(w2r, (2, 0, 1))     # rows r,    out re:  w2_re
    W2[1, :, :, 0, :] = -np.transpose(w2i, (2, 0, 1))    # rows 64+r, out re: -w2_im
    W2[0, :, :, 1, :] = np.transpose(w2i, (2, 0, 1))     # rows r,    out im:  w2_im
    W2[1, :, :, 1, :] = np.transpose(w2r, (2, 0, 1))     # rows 64+r, out im:  w2_re
    W2all = W2.reshape(128, 64 * 128).astype(F16)        # [c*64+r, l*128 + c'*64 + s]
    return np.ascontiguousarray(W1all), np.ascontiguousarray(W2all)


def build_bass(repeat=1):
    import concourse.bacc as bacc
    import concourse.mybir as mybir
    import concourse.tile as tile

    f16 = mybir.dt.float16
    f32 = mybir.dt.float32

    nc = bacc.Bacc("TRN2", target_bir_lowering=False)
    x = nc.dram_tensor("x", [B_CORE, N], f32, kind="ExternalInput")
    w1 = nc.dram_tensor("w1", [128, 64 * 128], f16, kind="ExternalInput")
    w2 = nc.dram_tensor("w2", [128, 64 * 128], f16, kind="ExternalInput")
    iddram = nc.dram_tensor("ident", [128, 128], f16, kind="ExternalInput")
    out = nc.dram_tensor("out", [64, 128, B_CORE], f16, kind="ExternalOutput")

    x_v = x[:, :].rearrange("(t h b0) (p k) -> t h p b0 k", h=2, b0=BT // 2, p=64)
    out_v = out[:, :, :].rearrange("L cs (t b) -> t cs L b", b=BT)

    with tile.TileContext(nc) as tc:
        with (
            tc.tile_pool(name="const", bufs=1) as constp,
            tc.tile_pool(name="t1", bufs=3) as t1p,
            tc.tile_pool(name="g", bufs=2) as gp,
            tc.tile_pool(name="t2s", bufs=6) as t2p,
            tc.tile_pool(name="outs", bufs=6) as outp,
            tc.tile_pool(name="po1", bufs=2, space="PSUM") as po1,
            tc.tile_pool(name="pt2", bufs=2, space="PSUM") as pt2,
            tc.tile_pool(name="po2", bufs=2, space="PSUM") as po2,
        ):
            # prefetch the first supertile's input before the weight tables
            T1_first = t1p.tile([128, (BT // 2) * 64], f16, tag="t1")
            T1f_4d = T1_first[:].rearrange("(h p) (b0 k) -> h p b0 k", h=2, k=64)
            for h in range(2):
                for kh in range(2):
                    nc.gpsimd.dma_start(
                        T1f_4d[h][:, :, kh * 32:(kh + 1) * 32],
                        x_v[0][h][:, :, kh * 32:(kh + 1) * 32])
            W1t = constp.tile([128, 64 * 128], f16)
            nc.sync.dma_start(W1t[:], w1[:, :])
            ident = constp.tile([128, 128], f16)
            nc.sync.dma_start(ident[:], iddram[:, :])
            W2t = constp.tile([128, 64 * 128], f16)
            nc.sync.dma_start(W2t[:], w2[:, :])
            W1t_v = W1t[:].rearrange("(h p) f -> h p f", h=2)

            from contextlib import nullcontext
            rep_ctx = tc.For_i(0, repeat, 1) if repeat > 1 else nullcontext()
            with rep_ctx:
                for t in range(NT):
                    # ---- load T1[(h,p), (b0,k)] with fp32->fp16 cast (SWDGE) ----
                    if t == 0 and repeat == 1:
                        T1_4d = T1f_4d
                    else:
                        T1 = t1p.tile([128, (BT // 2) * 64], f16, tag="t1")
                        T1_4d = T1[:].rearrange("(h p) (b0 k) -> h p b0 k", h=2, k=64)
                        for h in range(2):
                            for kh in range(2):
                                nc.gpsimd.dma_start(
                                    T1_4d[h][:, :, kh * 32:(kh + 1) * 32],
                                    x_v[t][h][:, :, kh * 32:(kh + 1) * 32])

                    # ---- stage 1 (data-stationary): G[b, ch, q*128+c*64+r] ----
                    G = gp.tile([128, 2, 64 * 128], f16)
                    G_5d = G[:].rearrange("B ch (q c r) -> B ch q c r", q=64, c=2)
                    for ch in range(2):
                        for kg2 in range(8):
                            O1 = po1.tile([128, 8, 128], f32)
                            for ksub in range(8):
                                k = kg2 * 8 + ksub
                                nc.tensor.matmul(
                                    O1[:, ksub, :],
                                    T1_4d[ch][:, :, k],                  # (64 p, 128 b0)
                                    W1t_v[ch][:, k * 128:(k + 1) * 128], # (64 p, 128 q2)
                                    start=True, stop=True,
                                )
                            # evac + cast f32->f16, (ksub,c,q)->(q,c,ksub)
                            src = O1[:].rearrange(
                                "B ksub (c q) -> B q c ksub", c=2)
                            dst = G_5d[:, ch, :, :, kg2 * 8:(kg2 + 1) * 8]
                            if kg2 % 4 == 3:
                                nc.vector.tensor_copy(dst, src)
                            else:
                                nc.scalar.copy(dst, src)

                    # ---- stage 2, l in quads: PE transposes -> T2s; pairs of mms ----
                    OUTS = None
                    T2s = None
                    for l0 in range(0, 64, 2):
                        grp = (t * 64 + l0) // 8
                        if l0 % 8 == 0:
                            OUTS = outp.tile([128, 8, BT], f16)
                        if l0 % 4 == 0:
                            Pt2 = pt2.tile([128, 8, 128], f16)
                            for lp in range(4):
                                l = l0 + lp
                                for ch in range(2):
                                    nc.tensor.transpose(
                                        Pt2[:, lp * 2 + ch, :],
                                        G[:, ch, l * 128:(l + 1) * 128], ident[:]
                                    )
                            T2s = t2p.tile([128, 4, 256], f16)
                            nc.vector.tensor_copy(T2s[:], Pt2[:])

                        O2 = po2.tile([128, 2, BT], f32)
                        for lp in range(2):
                            l = l0 + lp
                            nc.tensor.matmul(
                                O2[:, lp, :], W2t[:, l * 128:(l + 1) * 128],
                                T2s[:, l % 4, :],
                                start=True, stop=True,
                            )
                        if grp % 2 == 0:
                            nc.scalar.copy(OUTS[:, l0 % 8:l0 % 8 + 2, :], O2[:])
                        else:
                            nc.vector.tensor_copy(OUTS[:, l0 % 8:l0 % 8 + 2, :], O2[:])

                        if l0 % 8 == 6:
                            nc.sync.dma_start(out_v[t][:, l0 - 6:l0 + 2, :], OUTS[:])
    nc.compile()
    return nc


def _assemble_core(o: np.ndarray) -> np.ndarray:
    # o: (64 l, 128 cs, B_CORE) f16, cs = c*64+s  ->  (B_CORE, 4096) complex64
    a = o.reshape(64, 2, 64, B_CORE)                         # (l, c, s, b)
    a = np.ascontiguousarray(np.transpose(a, (3, 2, 0, 1)))  # (b, s, l, c)
    return a.astype(np.float32).view(np.complex64).reshape(B_CORE, N)


def kernel(x, w1_bfly, w2_bfly, perm, _trace=False):
    from concourse.bass_utils import run_bass_kernel_spmd

    x = np.asarray(x, dtype=np.float32)
    w1_bfly = np.asarray(w1_bfly, dtype=np.float32)
    w2_bfly = np.asarray(w2_bfly, dtype=np.float32)

    W1all, W2all = _build_host_weights(w1_bfly, w2_bfly)
    ident = np.eye(128, dtype=F16)
    nc = build_bass()
    in_maps = [
        {
            "x": np.ascontiguousarray(x[i * B_CORE:(i + 1) * B_CORE]),
            "w1": W1all,
            "w2": W2all,
            "ident": ident,
        }
        for i in range(NCORES)
    ]
    res = run_bass_kernel_spmd(
        nc, in_maps, core_ids=list(range(NCORES)), trace=_trace
    )
    outs = [_assemble_core(r["out"]) for r in res.results]
    full = np.concatenate(outs, axis=0)
    if _trace:
        return full, res
    return full



# revision 39
# speedup vs baseline: 45.1610x; 45.1610x over previous
"""Trainium2 Bass kernel for nn_ButterflyFFT (Monarch butterfly, N=4096, B=8192).

Math (per batch row b, viewing x[b] as a 64x64 matrix X with X[p,k]=x[b,p*64+k]):
  stage 1: for each column k: Y[:,k] = w1c[k] @ X[:,k]       (64x64 complex, X real)
  stage 2: for each row    l: Z[l,:] = w2c[l] @ Y[l,:]       (64x64 complex)
  output:  out[b, s*64+l] = Z[l,s]                            (complex64)

Device pipeline per core (B_core=1024, 8 supertiles of BU=128 batch rows):
  1. gather-DMA x -> T1[p, (b0,k)] fp16 (cast in SWDGE DMA), 2 chunks
  2. stage 1, data-stationary fp16 matmuls: out (b, q2) -> G[b, q*128+c*64+r]
  3. PE transpose per l: G-slice (b, (c r)) -> Pt2 (rc, b), octets of 8 l
  4. stage 2, weights-stationary fp16 matmuls: O2 (c's*64+s, b)
  5. OUTS assembles supertile PAIRS (256 b) so the out-DMA keeps 512B runs;
     DMA out fp16 (l, cs, b); host reassembles complex64.

Copy-engine budget per supertile (DMA-bound at ~11.7us):
  DVE:  8 T2s copies (f16 2x mode) + 4 G evacs + 2 OUTS
  Act:  12 G evacs + 4 OUTS
  Pool: input SWDGE descriptor gen + 2 OUTS (issued early in the l-loop)
"""

import numpy as np

N = 4096
B = 8192
NCORES = 8
B_CORE = B // NCORES  # 1024
BU = 128              # supertile batch rows
NU = B_CORE // BU     # 8 supertiles
F16 = np.float16


def _build_host_weights(w1_bfly: np.ndarray, w2_bfly: np.ndarray):
    """W1[p, k*128 + c*64 + q] = w1_bfly[k,q,p,c]
       W2all[c*64+r, l*128 + c'*64 + s] = stage-2 complex-matmul real form."""
    w1 = w1_bfly.astype(np.float32)              # (k, q, p, c)
    W1 = np.transpose(w1, (2, 0, 3, 1))          # (p, k, c, q)
    W1 = W1.reshape(64, 64 * 128).astype(F16)    # [p, k*128 + c*64 + q]

    w2r = w2_bfly[..., 0].astype(np.float32)     # (l, s, r)
    w2i = w2_bfly[..., 1].astype(np.float32)
    W2 = np.empty((2, 64, 64, 2, 64), dtype=np.float32)  # [c, r, l, c', s]
    W2[0, :, :, 0, :] = np.transpose(w2r, (2, 0, 1))     # rows r,    out re:  w2_re
    W2[1, :, :, 0, :] = -np.transpose(w2i, (2, 0, 1))    # rows 64+r, out re: -w2_im
    W2[0, :, :, 1, :] = np.transpose(w2i, (2, 0, 1))     # rows r,    out im:  w2_im
    W2[1, :, :, 1, :] = np.transpose(w2r, (2, 0, 1))     # rows 64+r, out im:  w2_re
    W2all = W2.reshape(128, 64 * 128).astype(F16)        # [c*64+r, l*128 + c'*64 + s]
    return np.ascontiguousarray(W1), np.ascontiguousarray(W2all)


def build_bass(repeat=1):
    import concourse.bacc as bacc
    import concourse.mybir as mybir
    import concourse.tile as tile

    f16 = mybir.dt.float16
    f32 = mybir.dt.float32

    nc = bacc.Bacc("TRN2", target_bir_lowering=False)
    x = nc.dram_tensor("x", [B_CORE, N], f32, kind="ExternalInput")
    w1 = nc.dram_tensor("w1", [64, 64 * 128], f16, kind="ExternalInput")
    w2 = nc.dram_tensor("w2", [128, 64 * 128], f16, kind="ExternalInput")
    iddram = nc.dram_tensor("ident", [128, 128], f16, kind="ExternalInput")
    out = nc.dram_tensor("out", [64, 128, B_CORE], f16, kind="ExternalOutput")

    x_v = x[:, :].rearrange("(u b0) (p k) -> u p b0 k", b0=BU, p=64)
    out_v = out[:, :, :].rearrange("L cs (t b) -> t cs L b", b=2 * BU)

    # PSUM can only be read by DVE/Act (GPSIMD/Pool is rejected by the BIR
    # verifier), so all evacs split across those two.
    # G evac engine per kg4 group (16 per supertile): DVE x4, Act x12.
    GEVAC_ENG = ["D", "A", "A", "A"] * 4
    # OUTS evac engine per octet (8 per supertile): DVE x3, Act x5.
    OUTS_ENG = ["A", "D", "A", "D", "A", "A", "D", "A"]

    with tile.TileContext(nc) as tc:
        with (
            tc.tile_pool(name="const", bufs=1) as constp,
            tc.tile_pool(name="t1", bufs=4) as t1p,
            tc.tile_pool(name="g", bufs=2) as gp,
            tc.tile_pool(name="t2s", bufs=4) as t2p,
            tc.tile_pool(name="outs", bufs=2) as outp,
            tc.tile_pool(name="po1", bufs=2, space="PSUM") as po1,
            tc.tile_pool(name="pt2", bufs=2, space="PSUM") as pt2,
            tc.tile_pool(name="po2", bufs=2, space="PSUM") as po2,
        ):
            def load_t1(u):
                """One 64-partition tile per supertile; 2 DMAs so SWDGE
                descriptor-gen overlaps the previous chunk's transfer."""
                T1u = t1p.tile([64, BU * 64], f16, tag="t1")
                T1_3d = T1u[:].rearrange("p (b0 k) -> p b0 k", k=64)
                for bq in range(2):
                    sl = slice(bq * 64, (bq + 1) * 64)
                    nc.gpsimd.dma_start(T1_3d[:, sl], x_v[u][:, sl])
                return T1u

            # W1 is the smallest gate for stage-1: load it over HWDGE while
            # the first input's SWDGE descriptors generate. ident/W2 go on the
            # Pool SWDGE queue so they transfer AFTER the first input (HWDGE
            # loads would otherwise grab the DMA bus first); W2 is split so
            # early stage-2 octets aren't gated on all of it.
            W1t = constp.tile([64, 64 * 128], f16)
            nc.sync.dma_start(W1t[:], w1[:, :])
            T1_first = load_t1(0)
            ident = constp.tile([128, 128], f16)
            nc.gpsimd.dma_start(ident[:], iddram[:, :])
            W2t = constp.tile([128, 64 * 128], f16)
            for wq in range(4):
                nc.gpsimd.dma_start(W2t[:, wq * 2048:(wq + 1) * 2048],
                                    w2[:, wq * 2048:(wq + 1) * 2048])

            from contextlib import nullcontext
            rep_ctx = tc.For_i(0, repeat, 1) if repeat > 1 else nullcontext()
            with rep_ctx:
                # T1 prefetched two supertiles ahead (bufs=6 covers 3 in
                # flight); OUTS tiles span supertile pairs for 512B out runs.
                def emit_s1_groups(T1_3d, G_4d, kg4_list):
                    """Stage-1 matmul groups + their evacs for the given kg4s."""
                    for kg4 in kg4_list:
                        O1 = po1.tile([128, 4, 128], f32)
                        for ksub in range(4):
                            k = kg4 * 4 + ksub
                            nc.tensor.matmul(
                                O1[:, ksub, :],
                                T1_3d[:, :, k],                  # (64 p, 128 b0)
                                W1t[:, k * 128:(k + 1) * 128],   # (64 p, 128 q2)
                                start=True, stop=True,
                            )
                        # evac + cast f32->f16, (ksub,c,q)->(q,c,ksub)
                        src = O1[:].rearrange("B ksub (c q) -> B q c ksub", c=2)
                        dst = G_4d[:, :, :, kg4 * 4:(kg4 + 1) * 4]
                        if GEVAC_ENG[kg4] == "D":
                            nc.vector.tensor_copy(dst, src)
                        else:
                            nc.scalar.copy(dst, src)

                pend = {}
                OUTS = None
                pending_out = []  # deferred (dst, src) out-DMAs, spread later
                G_cur = None
                for u in range(NU):
                    if u == 0:
                        T1u = T1_first if repeat == 1 else load_t1(0)
                        pend[1] = load_t1(1)
                        # fill: stage 1 of the first supertile runs standalone
                        G_cur = gp.tile([128, 64 * 128], f16)
                        emit_s1_groups(
                            T1u[:].rearrange("p (b0 k) -> p b0 k", k=64),
                            G_cur[:].rearrange("B (q c r) -> B q c r", q=64, c=2),
                            range(16))
                    if u + 2 < NU:
                        pend[u + 2] = load_t1(u + 2)

                    G = G_cur
                    # next supertile's stage 1 interleaves into this l-loop:
                    # 2 k-groups per octet, so its evac wall retires by the
                    # time this supertile's l-loop (and PE queue) drain.
                    if u + 1 < NU:
                        T1n = pend.pop(u + 1)
                        T1n_3d = T1n[:].rearrange("p (b0 k) -> p b0 k", k=64)
                        G_cur = gp.tile([128, 64 * 128], f16)
                        Gn_4d = G_cur[:].rearrange(
                            "B (q c r) -> B q c r", q=64, c=2)
                    else:
                        T1n_3d = None

                    # ---- stage 2, l in octets: 8 PE transposes -> T2s; 8 mms ----
                    half = u % 2
                    for l0 in range(0, 64, 8):
                        if half == 0 and l0 == 0:
                            OUTS = outp.tile([128, 8, 8, 2 * BU], f16)
                        Pt2 = pt2.tile([128, 8, 128], f16)
                        for lp in range(8):
                            l = l0 + lp
                            nc.tensor.transpose(
                                Pt2[:, lp, :],
                                G[:, l * 128:(l + 1) * 128], ident[:]
                            )
                        T2s = t2p.tile([128, 8, 128], f16)
                        nc.vector.tensor_copy(T2s[:], Pt2[:])

                        O2 = po2.tile([128, 8, BU], f32)
                        for lp in range(8):
                            l = l0 + lp
                            nc.tensor.matmul(
                                O2[:, lp, :], W2t[:, l * 128:(l + 1) * 128],
                                T2s[:, lp, :],
                                start=True, stop=True,
                            )
                        oct_i = l0 // 8
                        eng = OUTS_ENG[oct_i]
                        dst = OUTS[:, oct_i, :, half * BU:(half + 1) * BU]
                        if eng == "P":
                            nc.gpsimd.tensor_copy(dst, O2[:])
                        elif eng == "D":
                            nc.vector.tensor_copy(dst, O2[:])
                        else:
                            nc.scalar.copy(dst, O2[:])

                        if T1n_3d is not None:
                            emit_s1_groups(T1n_3d, Gn_4d,
                                           range(oct_i * 2, oct_i * 2 + 2))

                        # Out-DMA balancing: odd octets ship now; even octets
                        # are deferred into the next supertile's l-loop so DMA
                        # demand is ~even across supertiles instead of
                        # bursting 2x in the odd (pair-completing) ones.
                        if half == 1:
                            dma_args = (out_v[u // 2][:, l0:l0 + 8, :],
                                        OUTS[:, oct_i, :, :])
                            if oct_i % 2 == 1 or u == NU - 1:
                                nc.sync.dma_start(*dma_args)
                            else:
                                pending_out.append(dma_args)
                        elif pending_out and oct_i % 2 == 0:
                            nc.sync.dma_start(*pending_out.pop(0))
                for dma_args in pending_out:
                    nc.sync.dma_start(*dma_args)
    nc.compile()
    return nc


def _assemble_core(o: np.ndarray) -> np.ndarray:
    # o: (64 l, 128 cs, B_CORE) f16, cs = c*64+s  ->  (B_CORE, 4096) complex64
    a = o.reshape(64, 2, 64, B_CORE)                         # (l, c, s, b)
    a = np.ascontiguousarray(np.transpose(a, (3, 2, 0, 1)))  # (b, s, l, c)
    return a.astype(np.float32).view(np.complex64).reshape(B_CORE, N)


def kernel(x, w1_bfly, w2_bfly, perm, _trace=False):
    from concourse.bass_utils import run_bass_kernel_spmd

    x = np.asarray(x, dtype=np.float32)
    w1_bfly = np.asarray(w1_bfly, dtype=np.float32)
    w2_bfly = np.asarray(w2_bfly, dtype=np.float32)

    W1all, W2all = _build_host_weights(w1_bfly, w2_bfly)
    ident = np.eye(128, dtype=F16)
    nc = build_bass()
    in_maps = [
        {
            "x": np.ascontiguousarray(x[i * B_CORE:(i + 1) * B_CORE]),
            "w1": W1all,
            "w2": W2all,
            "ident": ident,
        }
        for i in range(NCORES)
    ]
    res = run_bass_kernel_spmd(
        nc, in_maps, core_ids=list(range(NCORES)), trace=_trace
    )
    outs = [_assemble_core(r["out"]) for r in res.results]
    full = np.concatenate(outs, axis=0)
    if _trace:
        return full, res
    return full


# revision 50
# speedup vs baseline: 45.9482x; 1.0174x over previous
"""Trainium2 Bass kernel for nn_ButterflyFFT (Monarch butterfly, N=4096, B=8192).

Math (per batch row b, viewing x[b] as a 64x64 matrix X with X[p,k]=x[b,p*64+k]):
  stage 1: for each column k: Y[:,k] = w1c[k] @ X[:,k]       (64x64 complex, X real)
  stage 2: for each row    l: Z[l,:] = w2c[l] @ Y[l,:]       (64x64 complex)
  output:  out[b, s*64+l] = Z[l,s]                            (complex64)

Device pipeline per core (B_core=1024, 8 supertiles of BU=128 batch rows):
  1. gather-DMA x -> T1[p, (b0,k)] fp16 (cast in SWDGE DMA), 2 chunks
  2. stage 1, data-stationary fp16 matmuls: out (b, q2) -> G[b, q*128+c*64+r]
  3. PE transpose per l: G-slice (b, (c r)) -> Pt2 (rc, b), octets of 8 l
  4. stage 2, weights-stationary fp16 matmuls: O2 (c's*64+s, b)
  5. OUTS assembles supertile PAIRS (256 b) so the out-DMA keeps 512B runs;
     DMA out fp16 (l, cs, b); host reassembles complex64.

Copy-engine budget per supertile (DMA-bound at ~11.7us):
  DVE:  8 T2s copies (f16 2x mode) + 4 G evacs + 2 OUTS
  Act:  12 G evacs + 4 OUTS
  Pool: input SWDGE descriptor gen + 2 OUTS (issued early in the l-loop)
"""

import numpy as np

N = 4096
B = 8192
NCORES = 8
B_CORE = B // NCORES  # 1024
BU = 128              # supertile batch rows
NU = B_CORE // BU     # 8 supertiles
F16 = np.float16


def _build_host_weights(w1_bfly: np.ndarray, w2_bfly: np.ndarray):
    """W1[p, k*128 + c*64 + q] = w1_bfly[k,q,p,c]
       W2all[c*64+r, l*128 + c'*64 + s] = stage-2 complex-matmul real form."""
    w1 = w1_bfly.astype(np.float32)              # (k, q, p, c)
    W1 = np.transpose(w1, (2, 0, 3, 1))          # (p, k, c, q)
    W1 = W1.reshape(64, 64 * 128).astype(F16)    # [p, k*128 + c*64 + q]

    w2r = w2_bfly[..., 0].astype(np.float32)     # (l, s, r)
    w2i = w2_bfly[..., 1].astype(np.float32)
    W2 = np.empty((2, 64, 64, 2, 64), dtype=np.float32)  # [c, r, l, c', s]
    W2[0, :, :, 0, :] = np.transpose(w2r, (2, 0, 1))     # rows r,    out re:  w2_re
    W2[1, :, :, 0, :] = -np.transpose(w2i, (2, 0, 1))    # rows 64+r, out re: -w2_im
    W2[0, :, :, 1, :] = np.transpose(w2i, (2, 0, 1))     # rows r,    out im:  w2_im
    W2[1, :, :, 1, :] = np.transpose(w2r, (2, 0, 1))     # rows 64+r, out im:  w2_re
    W2all = W2.reshape(128, 64 * 128).astype(F16)        # [c*64+r, l*128 + c'*64 + s]
    return np.ascontiguousarray(W1), np.ascontiguousarray(W2all)


def build_bass(repeat=1):
    import concourse.bacc as bacc
    import concourse.mybir as mybir
    import concourse.tile as tile

    f16 = mybir.dt.float16
    f32 = mybir.dt.float32

    nc = bacc.Bacc("TRN2", target_bir_lowering=False)
    x = nc.dram_tensor("x", [B_CORE, N], f32, kind="ExternalInput")
    w1 = nc.dram_tensor("w1", [64, 64 * 128], f16, kind="ExternalInput")
    w2 = nc.dram_tensor("w2", [128, 64 * 128], f16, kind="ExternalInput")
    iddram = nc.dram_tensor("ident", [128, 128], f16, kind="ExternalInput")
    out = nc.dram_tensor("out", [64, 128, B_CORE], f16, kind="ExternalOutput")

    x_v = x[:, :].rearrange("(u b0) (p k) -> u p b0 k", b0=BU, p=64)
    out_v = out[:, :, :].rearrange("L cs (t b) -> t cs L b", b=2 * BU)

    # PSUM can only be read by DVE/Act (GPSIMD/Pool is rejected by the BIR
    # verifier), so all evacs split across those two.
    # G evac engine per kg4 group (16 per supertile): DVE x4, Act x12.
    GEVAC_ENG = ["D", "A", "A", "A"] * 4
    GEVAC_FILL = GEVAC_ENG
    # OUTS evac engine per octet (8 per supertile): DVE x3, Act x5.
    OUTS_ENG = ["A", "D", "A", "D", "A", "A", "D", "A"]

    with tile.TileContext(nc) as tc:
        with (
            tc.tile_pool(name="const", bufs=1) as constp,
            tc.tile_pool(name="t1", bufs=4) as t1p,
            tc.tile_pool(name="g", bufs=2) as gp,
            tc.tile_pool(name="t2s", bufs=6) as t2p,
            tc.tile_pool(name="outs", bufs=2) as outp,
            tc.tile_pool(name="po1", bufs=2, space="PSUM") as po1,
            tc.tile_pool(name="pt2", bufs=2, space="PSUM") as pt2,
            tc.tile_pool(name="po2", bufs=2, space="PSUM") as po2,
        ):
            def load_t1(u):
                """One 64-partition tile per supertile. The fill supertiles
                (u<2) split by k so the interleaved stage-1 groups can start
                on chunk 0 (worth the 64B-run descriptor-floor cost there);
                later tiles arrive a full period early and load in b0-quarter
                chunks that interleave with out-DMAs at 128B runs."""
                T1u = t1p.tile([64, BU * 64], f16, tag="t1")
                T1_3d = T1u[:].rearrange("p (b0 k) -> p b0 k", k=64)
                for kq in range(2):
                    sl = slice(kq * 32, (kq + 1) * 32)
                    nc.gpsimd.dma_start(T1_3d[:, :, sl], x_v[u][:, :, sl])
                return T1u

            # W1 is the smallest gate for stage-1: load it over HWDGE while
            # the first input's SWDGE descriptors generate. ident/W2 go on the
            # Pool SWDGE queue so they transfer AFTER the first input (HWDGE
            # loads would otherwise grab the DMA bus first); W2 is split so
            # early stage-2 octets aren't gated on all of it.
            W1t = constp.tile([64, 64 * 128], f16)
            nc.sync.dma_start(W1t[:], w1[:, :])
            T1_first = load_t1(0)
            ident = constp.tile([128, 128], f16)
            nc.gpsimd.dma_start(ident[:], iddram[:, :])
            W2t = constp.tile([128, 64 * 128], f16)
            for wq in range(4):
                nc.gpsimd.dma_start(W2t[:, wq * 2048:(wq + 1) * 2048],
                                    w2[:, wq * 2048:(wq + 1) * 2048])

            from contextlib import nullcontext
            rep_ctx = tc.For_i(0, repeat, 1) if repeat > 1 else nullcontext()
            with rep_ctx:
                # T1 prefetched two supertiles ahead (bufs=6 covers 3 in
                # flight); OUTS tiles span supertile pairs for 512B out runs.
                def emit_s1_groups(T1_3d, G_4d, kg4_list, gevac=GEVAC_ENG):
                    """Stage-1 matmul groups + their evacs for the given kg4s."""
                    for kg4 in kg4_list:
                        O1 = po1.tile([128, 4, 128], f32)
                        for ksub in range(4):
                            k = kg4 * 4 + ksub
                            nc.tensor.matmul(
                                O1[:, ksub, :],
                                T1_3d[:, :, k],                  # (64 p, 128 b0)
                                W1t[:, k * 128:(k + 1) * 128],   # (64 p, 128 q2)
                                start=True, stop=True,
                            )
                        # evac + cast f32->f16, (ksub,c,q)->(q,c,ksub)
                        src = O1[:].rearrange("B ksub (c q) -> B q c ksub", c=2)
                        dst = G_4d[:, :, :, kg4 * 4:(kg4 + 1) * 4]
                        if gevac[kg4] == "D":
                            nc.vector.tensor_copy(dst, src)
                        else:
                            nc.scalar.copy(dst, src)

                pend = {}
                OUTS = None
                pending_out = []  # deferred (dst, src) out-DMAs, spread later
                G_cur = None
                for u in range(NU):
                    if u == 0:
                        T1u = T1_first if repeat == 1 else load_t1(0)
                        pend[1] = load_t1(1)
                        # fill: stage 1 of the first supertile runs standalone
                        G_cur = gp.tile([128, 64 * 128], f16)
                        emit_s1_groups(
                            T1u[:].rearrange("p (b0 k) -> p b0 k", k=64),
                            G_cur[:].rearrange("B (q c r) -> B q c r", q=64, c=2),
                            range(16), gevac=GEVAC_FILL)
                    if u + 2 < NU:
                        pend[u + 2] = load_t1(u + 2)

                    G = G_cur
                    # next supertile's stage 1 interleaves into this l-loop:
                    # 2 k-groups per octet, so its evac wall retires by the
                    # time this supertile's l-loop (and PE queue) drain.
                    if u + 1 < NU:
                        T1n = pend.pop(u + 1)
                        T1n_3d = T1n[:].rearrange("p (b0 k) -> p b0 k", k=64)
                        G_cur = gp.tile([128, 64 * 128], f16)
                        Gn_4d = G_cur[:].rearrange(
                            "B (q c r) -> B q c r", q=64, c=2)
                    else:
                        T1n_3d = None

                    # ---- stage 2, l in octets: 8 PE transposes -> T2s; 8 mms ----
                    half = u % 2
                    for l0 in range(0, 64, 8):
                        if half == 0 and l0 == 0:
                            OUTS = outp.tile([128, 8, 8, 2 * BU], f16)
                        Pt2 = pt2.tile([128, 8, 128], f16)
                        for lp in range(8):
                            l = l0 + lp
                            nc.tensor.transpose(
                                Pt2[:, lp, :],
                                G[:, l * 128:(l + 1) * 128], ident[:]
                            )
                        T2s = t2p.tile([128, 8, 128], f16)
                        nc.vector.tensor_copy(T2s[:], Pt2[:])

                        O2 = po2.tile([128, 8, BU], f32)
                        for lp in range(8):
                            l = l0 + lp
                            nc.tensor.matmul(
                                O2[:, lp, :], W2t[:, l * 128:(l + 1) * 128],
                                T2s[:, lp, :],
                                start=True, stop=True,
                            )
                        oct_i = l0 // 8
                        eng = OUTS_ENG[oct_i]
                        dst = OUTS[:, oct_i, :, half * BU:(half + 1) * BU]
                        if eng == "P":
                            nc.gpsimd.tensor_copy(dst, O2[:])
                        elif eng == "D":
                            nc.vector.tensor_copy(dst, O2[:])
                        else:
                            nc.scalar.copy(dst, O2[:])

                        if T1n_3d is not None:
                            emit_s1_groups(T1n_3d, Gn_4d,
                                           range(oct_i * 2, oct_i * 2 + 2))

                        # Out-DMA balancing: odd octets ship now; even octets
                        # are deferred into the next supertile's l-loop so DMA
                        # demand is ~even across supertiles instead of
                        # bursting 2x in the odd (pair-completing) ones.
                        if half == 1:
                            dma_args = (out_v[u // 2][:, l0:l0 + 8, :],
                                        OUTS[:, oct_i, :, :])
                            if oct_i % 2 == 1 or u == NU - 1:
                                nc.sync.dma_start(*dma_args)
                            else:
                                pending_out.append(dma_args)
                        elif pending_out and oct_i % 2 == 0:
                            nc.sync.dma_start(*pending_out.pop(0))
                for dma_args in pending_out:
                    nc.sync.dma_start(*dma_args)
    nc.compile()
    return nc


def _assemble_core(o: np.ndarray) -> np.ndarray:
    # o: (64 l, 128 cs, B_CORE) f16, cs = c*64+s  ->  (B_CORE, 4096) complex64
    a = o.reshape(64, 2, 64, B_CORE)                         # (l, c, s, b)
    a = np.ascontiguousarray(np.transpose(a, (3, 2, 0, 1)))  # (b, s, l, c)
    return a.astype(np.float32).view(np.complex64).reshape(B_CORE, N)


def kernel(x, w1_bfly, w2_bfly, perm, _trace=False):
    from concourse.bass_utils import run_bass_kernel_spmd

    x = np.asarray(x, dtype=np.float32)
    w1_bfly = np.asarray(w1_bfly, dtype=np.float32)
    w2_bfly = np.asarray(w2_bfly, dtype=np.float32)

    W1all, W2all = _build_host_weights(w1_bfly, w2_bfly)
    ident = np.eye(128, dtype=F16)
    nc = build_bass()
    in_maps = [
        {
            "x": np.ascontiguousarray(x[i * B_CORE:(i + 1) * B_CORE]),
            "w1": W1all,
            "w2": W2all,
            "ident": ident,
        }
        for i in range(NCORES)
    ]
    res = run_bass_kernel_spmd(
        nc, in_maps, core_ids=list(range(NCORES)), trace=_trace
    )
    outs = [_assemble_core(r["out"]) for r in res.results]
    full = np.concatenate(outs, axis=0)
    if _trace:
        return full, res
    return full
